# revision 1
# baseline (speedup 1.0000x reference)
"""Deformable 2D convolution (B=8, H=W=128, C=64, F=128, 3x3) for 8 Trainium2
NeuronCores, data-parallel over the batch dimension (one sample per core).

Per-core algorithm (all heavy math on the PE systolic array):
  1. offset conv as one 81-wide matmul pass over zero-padded x^T with an
     fp16 hi/lo residual split (fp32-accurate result), then per-tap shifts
     via small SBUF DMAs and an 81->9 selection matmul (hi/lo again).
     Offset precision matters: the reference bilinear clip is discontinuous
     at negative-integer sample positions.
  2. per (row, tap) the 1-D bilinear gather is a dense 128x128 interpolation
     matrix: a tent relu(1-|w-xi|) with fixed-point center xi = x0 + frac
     (u16, 1/512 steps), built in two 4x-mode tensor_scalar passes from a
     broadcast of xi.  The matmul applies min(|v|,1) = 1 - tent; the
     complement is removed exactly by a per-partition rowsum bias in the
     PSUM->SBUF copy (rowsums computed from the same fp16 x values).
  3. the 9-tap x 64-channel contraction is 5 accumulating matmuls per row
     (taps packed in pairs to K=128 via PSUM tile_position).
"""

import sys

sys.path.insert(0, "/opt/trn_rl_repo")

import numpy as np

import concourse.bass as bass
import concourse.bacc as bacc
import concourse.mybir as mybir
from concourse import tile
from concourse.tile_rust import add_dep_helper
from concourse.bass_utils import run_bass_kernel_spmd

F16 = np.float16
ALU = mybir.AluOpType
AFT = mybir.ActivationFunctionType
DT = mybir.dt

B = 8
H = 128
W = 128
C = 64
F = 128
T = 9  # taps
PW = W + 2  # padded row width (130)
NPAD = PW * PW  # 16900
XT_COLS = NPAD + 16  # slack so chunked views stay in bounds
CHW = 2080  # padded-grid columns consumed per offset chunk (16 rows)
CHALO = 2344  # chunk window incl. tap halo (2080 + 2*130 + 4)
BLK = 8  # output rows per tent block
NBLK = H // BLK  # 16
TFREE = BLK * T * W  # 9216 tent columns per block
N_GPS_BCAST = 16  # blocks whose xi broadcast runs on gpsimd (rest on DMA)
OUTB = 4  # output rows per store DMA

_BUILT = None
LAST_RESULT = None


def _ladder_barrier(tc, nc, fanin=1):
    """Full barrier with bounded per-instruction sem fan-in (HW wait-slot
    limits): chain of sync-engine nops, each waiting on `fanin` producers
    plus the previous nop.  Later instructions get a forward edge to the
    last nop via Tile's strict-barrier hook."""
    curr_bb = nc.cur_bb
    insts = [i for i in curr_bb.bb.instructions if i.is_executable()]
    start = getattr(tc, "_ladder_covered", 0)
    todo = insts[start:]
    prev = None
    if tc.barrier_instruction_and_bb is not None:
        prev = tc.barrier_instruction_and_bb[0]
    k = 0
    while k < len(todo) or prev is None:
        nop = nc.sync.nop()
        for j in todo[k : k + fanin]:
            add_dep_helper(nop.ins, j, reason="ladder")
        if prev is not None:
            add_dep_helper(nop.ins, prev, reason="ladder-chain")
        prev = nop.ins
        k += fanin
    tc.barrier_instruction_and_bb = (prev, curr_bb)
    tc._ladder_covered = len(curr_bb.bb.instructions)



def _build():
    nc = bacc.Bacc(None)

    xhi_d = nc.declare_dram_parameter("xhi", [H, W, C], DT.float16, isOutput=False)
    xhiT_d = nc.declare_dram_parameter("xhiT", [C, H * W], DT.float16, isOutput=False)
    xloT_d = nc.declare_dram_parameter("xloT", [C, H * W], DT.float16, isOutput=False)
    offw_d = nc.declare_dram_parameter("offw81", [C, 81], DT.float16, isOutput=False)
    offwl_d = nc.declare_dram_parameter("offw81l", [C, 81], DT.float16, isOutput=False)
    wpk_d = nc.declare_dram_parameter("wpk", [5, 128, F], DT.float16, isOutput=False)
    sel_d = nc.declare_dram_parameter("sel81", [81, T], DT.float16, isOutput=False)
    qs_d = nc.declare_dram_parameter("qscal", [72, 1], DT.float32, isOutput=False)
    cb_d = nc.declare_dram_parameter("convb", [F, 1], DT.float32, isOutput=False)
    jm_d = nc.declare_dram_parameter("jmat", [72, 2048], DT.float32, isOutput=False)
    iw_d = nc.declare_dram_parameter("iotaw", [128, 1], DT.float32, isOutput=False)
    id_d = nc.declare_dram_parameter("identh", [128, 128], DT.float16, isOutput=False)
    mk_d = nc.declare_dram_parameter("mask7f", [128, 1], DT.int16, isOutput=False)
    out_d = nc.declare_dram_parameter("out", [H, W, F], DT.float32, isOutput=True)

    xi_dram = nc.dram_tensor("xi_bounce", [H * T * W], DT.int16)

    with tile.TileContext(nc) as tc:
        with tc.tile_pool(name="cst", bufs=1) as cst:
            xw = cst.tile([128, H * C], DT.float16, tag="xw")
            offw81 = cst.tile([C, 81], DT.float16, tag="offw81")
            offw81l = cst.tile([C, 81], DT.float16, tag="offw81l")
            wpk = cst.tile([128, 5 * F], DT.float16, tag="wpk")
            sel81 = cst.tile([81, T], DT.float16, tag="sel81")
            qs = cst.tile([72, 1], DT.float32, tag="qs")
            cb = cst.tile([F, 1], DT.float32, tag="cb")
            jm = cst.tile([72, 2048], DT.float32, tag="jm")
            iw = cst.tile([128, 1], DT.float32, tag="iw")
            idh = cst.tile([128, 128], DT.float16, tag="idh")
            mk = cst.tile([128, 1], DT.int16, tag="mk")
            rsc = cst.tile([C, PW], DT.float32, tag="rsc")
            rspk = cst.tile([128, 5 * 128], DT.float32, tag="rspk")
            off72 = cst.tile([72, 2048], DT.float32, tag="off72")
            xq = cst.tile([72, 2048], DT.int16, tag="xq")

            nc.sync.dma_start(offw81[:], offw_d[:])
            nc.sync.dma_start(offw81l[:], offwl_d[:])
            nc.sync.dma_start(wpk[:].rearrange("p (h f) -> p h f", h=5),
                              wpk_d[:].rearrange("h p f -> p h f"))
            nc.sync.dma_start(sel81[:], sel_d[:])
            nc.sync.dma_start(qs[:], qs_d[:])
            nc.sync.dma_start(cb[:], cb_d[:])
            nc.sync.dma_start(jm[:], jm_d[:])
            nc.sync.dma_start(iw[:], iw_d[:])
            nc.sync.dma_start(idh[:], id_d[:])
            nc.sync.dma_start(mk[:], mk_d[:])
            # x row-major slabs [w, (r, c)]
            for g in range(8):
                nc.sync.dma_start(
                    xw[:, 16 * g * C : (16 * g + 16) * C].rearrange(
                        "w (r c) -> w r c", r=16
                    ),
                    xhi_d[16 * g : 16 * g + 16].rearrange("r w c -> w r c"),
                )

            # ------------- phase A/B/C: padded x^T, offsets, xi prep --------
            with tc.tile_pool(name="phAB", bufs=1) as ph:
                xpadT = ph.tile([C, XT_COLS], DT.float16, tag="xpadT")
                xpadTl = ph.tile([C, XT_COLS], DT.float16, tag="xpadTl")

                for xt in (xpadT, xpadTl):
                    nc.vector.memset(xt[:, 0:PW], 0.0)
                    nc.vector.memset(xt[:, (PW - 1) * PW : XT_COLS], 0.0)
                    nc.vector.memset(
                        xt[:, 0 : PW * PW].rearrange("c (r q) -> c r q", r=PW)[
                            :, 1 : PW - 1, 0:1
                        ],
                        0.0,
                    )
                    nc.vector.memset(
                        xt[:, 0 : PW * PW].rearrange("c (r q) -> c r q", r=PW)[
                            :, 1 : PW - 1, PW - 1 : PW
                        ],
                        0.0,
                    )
                for xt, src in ((xpadT, xhiT_d), (xpadTl, xloT_d)):
                    nc.sync.dma_start(
                        xt[:, 0 : PW * PW].rearrange("c (r q) -> c r q", r=PW)[
                            :, 1 : PW - 1, 1 : PW - 1
                        ],
                        src[:].rearrange("c (r w) -> c r w", r=H),
                    )

                _ladder_barrier(tc, nc)
                # row sums of fp16 x (fp32 accumulation) for the complement
                # bias; clip-pad the two edge columns.
                nc.vector.tensor_reduce(
                    rsc[:],
                    xpadT[:, 0 : PW * PW].rearrange("c (r q) -> c r q", r=PW),
                    mybir.AxisListType.X,
                    ALU.add,
                )
                nc.vector.tensor_copy(rsc[:, 0:1], rsc[:, 1:2])
                nc.vector.tensor_copy(rsc[:, PW - 1 : PW], rsc[:, PW - 2 : PW - 1])
                # rspk[(half,c), ch*128 + i] = rowsum[c, clip(i + p(tap) - 1)]
                for ch in range(5):
                    for half in range(2):
                        t = 2 * ch + half
                        if t >= T:
                            continue
                        p = t // 3
                        nc.sync.dma_start(
                            rspk[64 * half : 64 * half + 64, ch * 128 : (ch + 1) * 128],
                            rsc[:, p : p + 128],
                        )

                _ladder_barrier(tc, nc)
                # offset conv, chunked: 81-wide partials in fp32 PSUM with an
                # fp16 hi/lo residual split, then tap shifts + 81->9 reduce.
                with tc.tile_pool(name="poBp", bufs=1, space="PSUM") as poBp, \
                     tc.tile_pool(name="psOffp", bufs=1, space="PSUM") as psOffp, \
                     tc.tile_pool(name="scrp", bufs=2) as scrp, \
                     tc.tile_pool(name="stp", bufs=2) as stp, \
                     tc.tile_pool(name="off9p", bufs=2) as off9p:
                    for ci in range(8):
                        w0 = ci * CHW
                        poB = poBp.tile([81, CHALO], DT.float32, tag="poB")
                        for s0 in range(0, CHALO, 512):
                            ss = min(512, CHALO - s0)
                            nc.tensor.matmul(
                                poB[:, s0 : s0 + ss], offw81[:],
                                xpadT[:, w0 + s0 : w0 + s0 + ss],
                                start=True, stop=False,
                            )
                            nc.tensor.matmul(
                                poB[:, s0 : s0 + ss], offw81[:],
                                xpadTl[:, w0 + s0 : w0 + s0 + ss],
                                start=False, stop=False,
                            )
                            nc.tensor.matmul(
                                poB[:, s0 : s0 + ss], offw81l[:],
                                xpadT[:, w0 + s0 : w0 + s0 + ss],
                                start=False, stop=True,
                            )
                        scr32 = scrp.tile([81, CHALO], DT.float32, tag="scr32")
                        if ci % 2 == 0:
                            nc.scalar.activation(scr32[:], poB[:], AFT.Identity)
                        else:
                            nc.vector.tensor_copy(scr32[:], poB[:])
                        scrh = scrp.tile([81, CHALO], DT.float16, tag="scrh")
                        scrl = scrp.tile([81, CHALO], DT.float16, tag="scrl")
                        nc.gpsimd.tensor_copy(scrh[:], scr32[:])
                        nc.gpsimd.tensor_tensor(
                            scrl[:], scr32[:], scrh[:], op=ALU.subtract
                        )
                        sth = stp.tile([81, 2048], DT.float16, tag="sth")
                        stl = stp.tile([81, 2048], DT.float16, tag="stl")
                        for st, sc in ((sth, scrh), (stl, scrl)):
                            for pq in range(9):
                                off = (pq // 3) * PW + pq % 3
                                src = sc[
                                    pq * 9 : pq * 9 + 9, off : off + 16 * PW
                                ].rearrange("t (i j) -> t i j", i=16)[:, :, 0:128]
                                nc.sync.dma_start(
                                    st[pq * 9 : pq * 9 + 9, :].rearrange(
                                        "t (i j) -> t i j", i=16
                                    ),
                                    src,
                                )
                        for half in range(2):
                            poff = psOffp.tile([T, 1024], DT.float32, tag="poff")
                            for kk in range(2):
                                s0 = half * 1024 + kk * 512
                                nc.tensor.matmul(
                                    poff[:, kk * 512 : (kk + 1) * 512],
                                    sel81[:], sth[:, s0 : s0 + 512],
                                    start=True, stop=False,
                                )
                                nc.tensor.matmul(
                                    poff[:, kk * 512 : (kk + 1) * 512],
                                    sel81[:], stl[:, s0 : s0 + 512],
                                    start=False, stop=True,
                                )
                            off9 = off9p.tile([T, 1024], DT.float32, tag="off9")
                            if half == 0:
                                nc.vector.tensor_copy(off9[:], poff[:])
                            else:
                                nc.scalar.activation(off9[:], poff[:], AFT.Identity)
                            nc.sync.dma_start(
                                off72[ci * 9 : (ci + 1) * 9,
                                      half * 1024 : (half + 1) * 1024],
                                off9[:],
                            )

            # xi prep: xf -> floor/frac -> clip -> u16 fixed point (1/512)
            with tc.tile_pool(name="prep", bufs=1) as pp:
                xf = pp.tile([72, 2048], DT.float32, tag="xf")
                t1 = pp.tile([72, 2048], DT.float32, tag="t1")
                ti = pp.tile([72, 2048], DT.int32, tag="ti")
                x0f = pp.tile([72, 2048], DT.float32, tag="x0f")
                x0c = pp.tile([72, 2048], DT.float32, tag="x0c")
                w1 = pp.tile([72, 2048], DT.float32, tag="w1")
                mm = pp.tile([72, 2048], DT.float32, tag="mm")
                w1s = pp.tile([72, 2048], DT.float32, tag="w1s")
                xif = pp.tile([72, 2048], DT.float32, tag="xif")

                nc.vector.scalar_tensor_tensor(
                    xf[:], off72[:], qs[:, 0:1], jm[:], op0=ALU.add, op1=ALU.add
                )
                # int32 conversion: truncation (sim) or round-to-nearest (hw).
                # +16 then a compare-fixup gives an exact floor either way.
                nc.vector.tensor_scalar(t1[:], xf[:], 16.0, 0.0, op0=ALU.add, op1=ALU.add)
                nc.vector.tensor_copy(ti[:], t1[:])
                nc.vector.tensor_scalar(x0f[:], ti[:], -16.0, 0.0, op0=ALU.add, op1=ALU.add)
                fixg = pp.tile([72, 2048], DT.float32, tag="fixg")
                nc.vector.tensor_tensor(fixg[:], x0f[:], xf[:], op=ALU.is_gt)
                nc.vector.tensor_tensor(x0f[:], x0f[:], fixg[:], op=ALU.subtract)
                nc.vector.tensor_scalar(x0c[:], x0f[:], 0.0, 127.0, op0=ALU.max, op1=ALU.min)
                nc.vector.tensor_tensor(w1[:], xf[:], x0f[:], op=ALU.subtract)
                nc.vector.tensor_scalar(mm[:], x0c[:], 126.5, 0.0, op0=ALU.is_le, op1=ALU.add)
                nc.vector.scalar_tensor_tensor(
                    w1s[:], w1[:], 512.0, mm[:], op0=ALU.mult, op1=ALU.mult
                )
                nc.vector.scalar_tensor_tensor(
                    xif[:], x0c[:], 512.0, w1s[:], op0=ALU.mult, op1=ALU.add
                )
                nc.vector.tensor_scalar(
                    xif[:], xif[:], -32768.0, 0.0, op0=ALU.add, op1=ALU.add
                )
                nc.vector.tensor_copy(xq[:], xif[:])

            # reorder xi into (i, t, j) order in DRAM, one block at a time
            for bi in range(NBLK):
                src = xq[(bi // 2) * 9 : (bi // 2) * 9 + 9,
                         (bi % 2) * 1024 : (bi % 2) * 1024 + 1024].rearrange(
                    "t (k j) -> t k j", k=BLK
                )
                dst = xi_dram[bi * TFREE : (bi + 1) * TFREE].rearrange(
                    "(k t j) -> t k j", k=BLK, t=T
                )
                nc.gpsimd.dma_start(dst, src)

            _ladder_barrier(tc, nc)
            # ---------------- steady state: tents, sampling, contraction ----
            with tc.tile_pool(name="tents", bufs=2) as tp, \
                 tc.tile_pool(name="row0p", bufs=2) as rp, \
                 tc.tile_pool(name="samp", bufs=4) as sp, \
                 tc.tile_pool(name="outp", bufs=3) as op_, \
                 tc.tile_pool(name="psS", bufs=2, space="PSUM") as psS, \
                 tc.tile_pool(name="psO", bufs=2, space="PSUM") as psO, \
                 tc.tile_pool(name="psT", bufs=2, space="PSUM") as psT:
                ptile = None
                for bi in range(NBLK):
                    xib = tp.tile([128, TFREE], DT.int16, tag="xib")
                    sl = xi_dram[bi * TFREE : (bi + 1) * TFREE]
                    # seed partition 0, then log2-double across partitions
                    nc.gpsimd.dma_start(
                        xib[0:1, :], sl.rearrange("(o f) -> o f", o=1)
                    )
                    npart = 1
                    while npart < 128:
                        eng = nc.sync if npart % 2 == 0 else nc.gpsimd
                        eng.dma_start(
                            xib[npart : 2 * npart, :], xib[0:npart, :]
                        )
                        npart *= 2
                    vt = tp.tile([128, TFREE], DT.float16, tag="vt")
                    nc.vector.tensor_scalar(
                        vt[:], xib[:], iw[:, 0:1], 512.0,
                        op0=ALU.add, op1=ALU.min,
                    )
                    nc.vector.tensor_scalar(
                        vt[:], vt[:], -512.0, 0.0, op0=ALU.max, op1=ALU.bypass
                    )
                    vti = vt[:].bitcast(DT.int16)
                    nc.vector.add_instruction(mybir.InstTensorScalarPtr(
                        name=nc.get_next_instruction_name(),
                        is_scalar_tensor_tensor=False,
                        op0=ALU.bitwise_and, op1=ALU.bypass,
                        ins=[nc.vector.lower_ap(vti),
                             mybir.ImmediateValue(dtype=DT.int32, value=32767),
                             mybir.ImmediateValue(dtype=DT.float32, value=0.0)],
                        outs=[nc.vector.lower_ap(vti)]))

                    for k in range(BLK):
                        i = bi * BLK + k
                        ps = psS.tile([128, 5 * 128], DT.float32, tag="ps")
                        for t in range(T):
                            p = t // 3
                            r = min(max(i + p - 1, 0), H - 1)
                            ch, half = t // 2, t % 2
                            nc.tensor.matmul(
                                ps[64 * half : 64 * half + 64, ch * 128 : (ch + 1) * 128],
                                xw[:, r * C : (r + 1) * C],
                                vt[:, (k * T + t) * 128 : (k * T + t + 1) * 128],
                                start=True, stop=True,
                                tile_position=(0, 64 * half),
                            )
                        ssb = sp.tile([128, 5 * 128], DT.float16, tag="ssb")
                        for ch in range(5):
                            hp = 128 if ch < 4 else 64  # tap 8 fills lower half only
                            nc.scalar.activation(
                                ssb[0:hp, ch * 128 : (ch + 1) * 128],
                                ps[0:hp, ch * 128 : (ch + 1) * 128],
                                AFT.Identity,
                                bias=rspk[0:hp, ch * 128 + i : ch * 128 + i + 1],
                                scale=-1.0 / 512.0,
                            )
                        po = psO.tile([F, 128], DT.float32, tag="po")
                        for ch in range(4):
                            nc.tensor.matmul(
                                po[:],
                                wpk[:, ch * 128 : (ch + 1) * 128],
                                ssb[:, ch * 128 : (ch + 1) * 128],
                                start=(ch == 0), stop=False,
                            )
                        nc.tensor.matmul(
                            po[:],
                            wpk[0:64, 4 * 128 : 5 * 128],
                            ssb[0:64, 4 * 128 : 5 * 128],
                            start=False, stop=True,
                        )
                        osb = op_.tile([F, 128], DT.float16, tag="osb")
                        nc.scalar.activation(
                            osb[:], po[:], AFT.Identity, bias=cb[:, 0:1], scale=1.0
                        )
                        if i % OUTB == 0:
                            ptile = psT.tile([128, OUTB * 128], DT.float16, tag="ptile")
                        nc.tensor.transpose(
                            ptile[:, (i % OUTB) * 128 : (i % OUTB + 1) * 128], osb[:], idh[:]
                        )
                        if i % OUTB == OUTB - 1:
                            i0 = i - (OUTB - 1)
                            otile = op_.tile([128, OUTB * 128], DT.float32, tag="otile")
                            nc.scalar.activation(otile[:], ptile[:], AFT.Identity)
                            nc.sync.dma_start(
                                out_d[i0 : i0 + OUTB].rearrange("i j f -> j i f"),
                                otile[:].rearrange("p (q f) -> p q f", q=OUTB),
                            )
    nc.finalize()
    return nc


def _host_pack(offset_W, offset_b, conv_W):
    offw81_32 = np.zeros((C, 81), dtype=np.float32)
    for p in range(3):
        for q in range(3):
            pq = 3 * p + q
            offw81_32[:, pq * 9 : pq * 9 + 9] = offset_W[p, q]  # [C, 9]
    offw81 = offw81_32.astype(F16)
    offw81l = (offw81_32 - offw81.astype(np.float32)).astype(F16)
    sel81 = np.zeros((81, T), dtype=np.float32)
    for pq in range(9):
        for t in range(T):
            sel81[pq * 9 + t, t] = 1.0
    wpk = np.zeros((5, 128, F), dtype=np.float32)
    for t in range(T):
        p, q = t // 3, t % 3
        ch, half = t // 2, t % 2
        wpk[ch, 64 * half : 64 * half + 64, :] = conv_W[p, q]  # [C, F]
    qscal = np.zeros((72, 1), dtype=np.float32)
    for ih in range(8):
        for t in range(T):
            q = t % 3
            qscal[ih * 9 + t, 0] = (q - 1) + offset_b[t]
    jmat = np.tile(np.arange(W, dtype=np.float32), (72, 16)).reshape(72, 2048)
    iotaw = (512.0 * (64.0 - np.arange(128, dtype=np.float32))).reshape(128, 1)
    identh = np.eye(128, dtype=F16)
    return {
        "offw81": offw81,
        "offw81l": offw81l,
        "wpk": wpk.astype(F16),
        "sel81": sel81.astype(F16),
        "qscal": qscal,
        "jmat": jmat,
        "iotaw": iotaw,
        "identh": identh,
        "mask7f": np.full((128, 1), 32767, dtype=np.int16),
    }


def kernel(x_in, offset_W, offset_b, conv_W, conv_b):
    global _BUILT
    x_in = np.asarray(x_in, dtype=np.float32)
    offset_W = np.asarray(offset_W, dtype=np.float32)
    offset_b = np.asarray(offset_b, dtype=np.float32)
    conv_W = np.asarray(conv_W, dtype=np.float32)
    conv_b = np.asarray(conv_b, dtype=np.float32)

    shared = _host_pack(offset_W, offset_b, conv_W)
    shared["convb"] = conv_b.reshape(F, 1).astype(np.float32)

    if _BUILT is None:
        _BUILT = _build()
    nc = _BUILT

    in_maps = []
    for b in range(B):
        xb = x_in[b]
        xhi = xb.astype(F16)
        xlo = (xb - xhi.astype(np.float32)).astype(F16)
        xhiT = np.ascontiguousarray(xhi.transpose(2, 0, 1).reshape(C, H * W))
        xloT = np.ascontiguousarray(xlo.transpose(2, 0, 1).reshape(C, H * W))
        in_maps.append(
            {"xhi": np.ascontiguousarray(xhi), "xhiT": xhiT, "xloT": xloT, **shared}
        )
    res = run_bass_kernel_spmd(nc, in_maps, list(range(B)))
    global LAST_RESULT
    LAST_RESULT = res
    out = np.stack([res.results[b]["out"] for b in range(B)], axis=0)
    return out.astype(np.float32)


if __name__ == "__main__":
    rng = np.random.default_rng(0)
    x = rng.standard_normal((B, H, W, C), dtype=np.float32)
    oW = rng.standard_normal((3, 3, C, 9), dtype=np.float32) * 0.05
    ob = rng.standard_normal((9,), dtype=np.float32) * 0.05
    cW = rng.standard_normal((3, 3, C, F), dtype=np.float32) / np.sqrt(9 * C)
    cb = rng.standard_normal((F,), dtype=np.float32) * 0.01
    y = kernel(x, oW, ob, cW, cb)
    print(y.shape, y.dtype)

